# revision 20
# baseline (speedup 1.0000x reference)
"""Multi-head causal attention (B=4, S=2048, E=1024, H=16, D=64) on 8 trn2 cores.

Sharding: core c handles batch b = c//2 and head-group g = c%2 (8 heads each).
Each core computes its partial output projection over its 512 local concat
columns; the host sums the two partials per batch and adds bp.

Math simplifications (exact):
  - bk is dropped: (q+bq)@bk is constant over the key index, so it cancels
    in the softmax.
  - bv is dropped on-device: softmax weights sum to 1, so bv passes through
    attention unchanged; the host adds bv_flat @ Wp into the bp term.

Layout strategy (per core):
  - x pre-transposed on host: xT [8, 128, S] (e on partitions).
  - Q^T, K^T computed as [d, s] (d on partitions, 2 heads per 128-partition
    pair tile) so scores come out transposed: scoresT [t, s]. The two heads'
    score matmuls are K=64 row-tiled (tile_position auto-derived) and run
    concurrently on the two halves of the PE array.
  - V kept natural [t, d] with a ones column appended per head (padded to 66
    cols), so the PV matmul also produces the softmax denominator as row 64.
  - Softmax: exp on ACT (no max subtraction -- scores are O(1)), causal
    masking via one multiplicative [128,512] (u>=p) mask on DVE, denominator
    pair broadcast across partitions via one K=2 selector matmul, reciprocal
    on DVE, normalize into concat^T, output projection from concat^T.

Schedule strategy:
  - DMA priority: wq/wk are split per head-pair r so the first projection
    matmuls start ~4us in; 20 dummy warmup matmuls keep the PE HAM clock
    gate at full rate while inputs stream.
  - scores->exp->PV is software-pipelined 2 blocks deep so the ACT exp
    latency never stalls the PE queue head; proj is emitted in half-items
    as filler.
"""

import numpy as np

B, S, E, H, D = 4, 2048, 1024, 16, 64
NCORES = 8
PAIRS = 4  # head pairs per core (8 heads)
ET = 8  # e-tiles of 128 for the contraction over E
SCH = 4  # s-chunks of 512
VW = 66  # V columns per head: 64 d + 1 ones + 1 pad
SCALE = float(D) ** -0.5
WARMUP_MMS = 10

MM_DTYPE = "bfloat16"
PV_FP8 = True  # fp8e4 DoubleRow for the PV matmuls (2 t-blocks per matmul)
DEBUG_DUMPS = False

_CACHE = {}


def host_round(a):
    a = np.ascontiguousarray(a, np.float32)
    if MM_DTYPE == "bfloat16":
        import ml_dtypes

        return a.astype(ml_dtypes.bfloat16)
    return a


def _build():
    import concourse.tile as tile
    from concourse import bacc, mybir
    from contextlib import ExitStack

    f32 = mybir.dt.float32
    mdt = getattr(mybir.dt, MM_DTYPE)
    f8 = mybir.dt.float8e4
    pdt = f8 if PV_FP8 else mdt
    AF = mybir.ActivationFunctionType

    nc = bacc.Bacc("TRN2", target_bir_lowering=False, debug=False, num_devices=NCORES)

    xt_d = nc.dram_tensor("xt", [8, 128, S], mdt, kind="ExternalInput").ap()
    wq_d = nc.dram_tensor("wq", [128, PAIRS, ET, 128], mdt, kind="ExternalInput").ap()
    wk_d = nc.dram_tensor("wk", [128, PAIRS, ET, 128], mdt, kind="ExternalInput").ap()
    wv_d = nc.dram_tensor("wv", [128, ET, 512], mdt, kind="ExternalInput").ap()
    wp_d = nc.dram_tensor("wp", [128, PAIRS, E], mdt, kind="ExternalInput").ap()
    bq_d = nc.dram_tensor("bq", [128, PAIRS], f32, kind="ExternalInput").ap()
    mask_d = nc.dram_tensor("mask", [128, 512], mdt, kind="ExternalInput").ap()
    dumm_d = nc.dram_tensor("dumm", [128, 640], mdt, kind="ExternalInput").ap()
    sel_d = nc.dram_tensor("sel", [65, 128], mdt, kind="ExternalInput").ap()
    y_d = nc.dram_tensor("y", [S, E], f32, kind="ExternalOutput").ap()
    if DEBUG_DUMPS:
        dbg_dn = nc.dram_tensor("dbg_dn", [65, 512], f32,
                                kind="ExternalOutput").ap()
        dbg_d = {
            nm: nc.dram_tensor(f"dbg_{nm}", shp, mdt, kind="ExternalOutput").ap()
            for nm, shp in [
                ("qt", [128, PAIRS, S]), ("kt", [128, PAIRS, S]),
                ("va", [128, 16, 8 * VW]), ("cat", [128, PAIRS, S]),
                ("rdb", [65, 512]), ("mask", [128, 512]),
            ]
        }

    with tile.TileContext(nc) as tc, ExitStack() as ctx:
        pers = ctx.enter_context(tc.tile_pool(name="pers", bufs=1))
        work = ctx.enter_context(tc.tile_pool(name="work", bufs=1))
        psp = ctx.enter_context(tc.tile_pool(name="psp", bufs=1, space="PSUM"))

        qt = pers.tile([128, PAIRS, S], mdt)  # Q^T pair tiles
        kt = pers.tile([128, PAIRS, S], mdt)  # K^T pair tiles
        # V (+ones col), indexed [pair, ko]; chunk-0 PV runs in bf16 (short
        # prefixes can't average out fp8 noise), chunks 1-3 in fp8 DoubleRow
        if PV_FP8:
            va8 = pers.tile([128, 8, 2, 8 * VW], f8)
            va16 = pers.tile([128, 2, 2, 8 * VW], mdt)
        else:
            va16 = pers.tile([128, 8, 2, 8 * VW], mdt)
            va8 = None
        cat = pers.tile([128, PAIRS, S], mdt)  # concat^T
        bq_sb = pers.tile([128, PAIRS], f32)
        sel = pers.tile([65, 128], mdt)  # denom pair-broadcast selector
        dn = pers.tile([65, 512], f32)  # denoms staged at partitions 0 / 64
        rd32 = pers.tile([65, 512], f32)
        rdb = pers.tile([65, 512], mdt)
        wq_sb = pers.tile([128, PAIRS, ET, 128], mdt)
        wk_sb = pers.tile([128, PAIRS, ET, 128], mdt)
        wv_sb = pers.tile([128, ET, 512], mdt)
        wp_sb = pers.tile([128, PAIRS, E], mdt)
        mask_sb = pers.tile([128, 512], mdt)
        mask8 = pers.tile([128, 512], f8, name="mask8") if PV_FP8 else None
        dumm = pers.tile([128, 640], mdt)
        xts = [work.tile([128, ET, 512], mdt, tag=f"xt{j}", name=f"xt{j}")
               for j in range(SCH)]

        # memsets first: no deps, run immediately
        nc.vector.memset(dn, 1.0)
        nc.vector.memset(rd32, 0.0)
        nc.vector.memset(rdb, 0.0)

        xt_view = xt_d.rearrange("e p s -> p e s")
        # DMA priority order. sync queue: what the first Q matmuls need;
        # gpsimd queue: K weights then the bulk.
        nc.sync.dma_start(out=dumm, in_=dumm_d)
        nc.sync.dma_start(out=wq_sb[:, 0], in_=wq_d[:, 0])
        nc.sync.dma_start(out=xts[0], in_=xt_view[:, :, 0:512])
        nc.gpsimd.dma_start(out=bq_sb, in_=bq_d)
        nc.gpsimd.dma_start(out=sel, in_=sel_d)
        # guard: a writer into wk_sb that depends on xt0 holds the wk DMA
        # (WAW) and the rest of the gpsimd FIFO until the critical xt0 lands,
        # so the first Q matmuls get the full HBM bandwidth.
        nc.vector.tensor_copy(wk_sb[0:1, 0, 0, 0:2], xts[0][0:1, 0, 0:2])
        nc.gpsimd.dma_start(out=wk_sb[:, 0], in_=wk_d[:, 0])
        for r in range(1, PAIRS):
            nc.sync.dma_start(out=wq_sb[:, r], in_=wq_d[:, r])
            nc.gpsimd.dma_start(out=wk_sb[:, r], in_=wk_d[:, r])
        nc.sync.dma_start(out=mask_sb, in_=mask_d)
        if PV_FP8:
            nc.vector.tensor_copy(mask8, mask_sb)
        nc.gpsimd.dma_start(out=wv_sb, in_=wv_d)
        for j in range(1, SCH):
            nc.sync.dma_start(out=xts[j], in_=xt_view[:, :, j * 512:(j + 1) * 512])
        nc.gpsimd.dma_start(out=wp_sb, in_=wp_d)

        # PE warmup: keep the HAM clock gate hot while inputs stream.
        # One accumulation group -> no WAR semaphores between members, so
        # the matmuls stream back-to-back (a full HAM busy window).
        wps = psp.tile([128, 512], f32, tag="mm512", bufs=2, name="warm")
        for i in range(WARMUP_MMS):
            nc.tensor.matmul(wps, lhsT=dumm[:, 0:128], rhs=dumm[:, 128:640],
                             start=(i == 0), stop=(i == WARMUP_MMS - 1))

        # ---------------- work-item emitters ----------------
        def emit_warm():
            # 2 full-array dummy matmuls: soak PE idle in ACT-bound or
            # DMA-bound stretches so the HAM clock gate stays at 8/8
            w = psp.tile([128, 512], f32, tag="mm512", bufs=2, name="keep")
            for i in range(2):
                nc.tensor.matmul(w, lhsT=dumm[:, 0:128], rhs=dumm[:, 128:640],
                                 start=(i == 0), stop=(i == 1))

        def emit_q(j, r):
            sjl = slice(j * 512, (j + 1) * 512)
            ps = psp.tile([128, 512], f32, tag="mm512", bufs=2, name="qps")
            for et in range(ET):
                nc.tensor.matmul(ps, lhsT=wq_sb[:, r, et], rhs=xts[j][:, et],
                                 start=(et == 0), stop=(et == ET - 1))
            nc.vector.tensor_scalar_add(qt[:, r, sjl], ps, bq_sb[:, r:r + 1])

        def emit_k(j, r):
            sjl = slice(j * 512, (j + 1) * 512)
            ps = psp.tile([128, 512], f32, tag="mm512", bufs=2, name="kps")
            for et in range(ET):
                nc.tensor.matmul(ps, lhsT=wk_sb[:, r, et], rhs=xts[j][:, et],
                                 start=(et == 0), stop=(et == ET - 1))
            nc.vector.tensor_copy(kt[:, r, sjl], ps)

        def emit_v(j, ii):
            i = 4 * j + ii
            si = slice(ii * 128, (ii + 1) * 128)
            ps = psp.tile([128, 512], f32, tag="mm512", bufs=2, name="vps")
            for et in range(ET):
                nc.tensor.matmul(ps, lhsT=xts[j][:, et, si], rhs=wv_sb[:, et],
                                 start=(et == 0), stop=(et == ET - 1))
            tgts = []
            if PV_FP8:
                tgts.append(va8[:, i // 2, i % 2, :])
                if j == 0:
                    tgts.append(va16[:, i // 2, i % 2, :])
            else:
                tgts.append(va16[:, i // 2, i % 2, :])
            for t in tgts:
                va_i = t.rearrange("p (h c) -> p h c", c=VW)
                nc.vector.tensor_copy(
                    va_i[:, :, 0:64], ps.rearrange("p (h d) -> p h d", d=64)
                )
                nc.vector.memset(va_i[:, :, 64:65], 1.0)
                nc.vector.memset(va_i[:, :, 65:66], 0.0)

        attn_pr = {}
        attn_out = {}

        def emit_sc(j, r, ti):
            v = max(ti - 4 * j, 0)
            sjv = slice(j * 512 + 128 * v, (j + 1) * 512)
            tis = slice(ti * 128, (ti + 1) * 128)
            scp = psp.tile([128, 2, 512], f32, tag="sc", bufs=2)
            for hh in range(2):
                po = hh * 64
                nc.tensor.matmul(
                    scp[:, hh, 128 * v:],
                    lhsT=kt[po:po + 64, r, tis],
                    rhs=qt[po:po + 64, r, sjv],
                    start=True, stop=True,
                )
            tp, ko = ti // 2, ti % 2
            fp8c = PV_FP8 and j > 0
            if ko == 0:
                if fp8c:
                    attn_pr[(j, r, tp)] = work.tile(
                        [128, 2, 2, 512], f8, tag="pr", bufs=6, name="pr8"
                    )
                else:
                    attn_pr[(j, r, tp)] = work.tile(
                        [128, 2, 2, 512], mdt, tag="pr16", bufs=3, name="pr16"
                    )
            pr = attn_pr[(j, r, tp)]
            nc.scalar.activation(
                pr[:, :, ko, 128 * v:], scp[:, :, 128 * v:], AF.Exp,
                scale=SCALE
            )
            if v or ti == 4 * j:
                msk = mask8 if fp8c else mask_sb
                for hh in range(2):
                    nc.vector.tensor_mul(
                        pr[:, hh, ko, 128 * v:],
                        pr[:, hh, ko, 128 * v:],
                        msk[:, 0:512 - 128 * v],
                    )
            if fp8c and ko == 1 and v >= 1:
                # diagonal pair hole: the odd block's [128(v-1), 128v) slice
                # is streamed by the pair matmul but must contribute zero
                nc.vector.memset(pr[:, :, 1, 128 * (v - 1):128 * v], 0.0)

        def emit_pvp(j, r, tp):
            np2 = 2 * j + 2
            if tp == 0:
                attn_out[(j, r)] = [
                    psp.tile([VW, 512], f32, tag=f"o{hh}", bufs=1,
                             name=f"outp{hh}")
                    for hh in range(2)
                ]
            outps = attn_out[(j, r)]
            pr = attn_pr.pop((j, r, tp))
            vA = max(2 * tp - 4 * j, 0)
            fp8c = PV_FP8 and j > 0
            for hh in range(2):
                h = 2 * r + hh
                if fp8c:
                    nc.tensor.matmul(
                        outps[hh][:, 128 * vA:],
                        lhsT=va8[:, tp, :, h * VW:(h + 1) * VW],
                        rhs=pr[:, hh, :, 128 * vA:],
                        perf_mode=mybir.MatmulPerfMode.DoubleRow,
                        start=(tp == 0), stop=(tp == np2 - 1),
                    )
                else:
                    for ko in range(2):
                        v = max(2 * tp + ko - 4 * j, 0)
                        nc.tensor.matmul(
                            outps[hh][:, 128 * v:],
                            lhsT=va16[:, tp, ko, h * VW:(h + 1) * VW],
                            rhs=pr[:, hh, ko, 128 * v:],
                            start=(tp == 0 and ko == 0),
                            stop=(tp == np2 - 1 and ko == 1),
                        )

        def emit_norm(j, r):
            outps = attn_out.pop((j, r))
            sjl = slice(j * 512, (j + 1) * 512)
            # PSUM evacuation first: releases the o-banks for the next
            # head pair's PV accumulation as early as possible
            osbs = []
            for hh in range(2):
                osb = work.tile([65, 512], mdt, tag="osb", bufs=4)
                nc.vector.tensor_copy(osb, outps[hh][0:65, :])
                osbs.append(osb)
            # denominators (row 64) -> reciprocal -> pair-broadcast matmul.
            # Full-tile DVE ops from base partition 0: custom-DVE ops
            # misbehave at base partition 64 on HW (rows 1-63 are 1.0 and
            # sel zeroes them in the matmul).
            for hh, po in ((0, 0), (1, 64)):
                nc.vector.tensor_copy(dn[po:po + 1], osbs[hh][64:65, :])
            nc.vector.reciprocal_approx_fast(rd32, dn)
            nc.vector.tensor_copy(rdb, rd32)
            bcp = psp.tile([128, 512], f32, tag="mm512", bufs=2, name="bcst")
            nc.tensor.matmul(bcp, lhsT=sel, rhs=rdb, start=True, stop=True)
            for hh in range(2):
                po = hh * 64
                nc.vector.tensor_mul(
                    cat[po:po + 64, r, sjl], osbs[hh][0:64, :],
                    bcp[po:po + 64, :]
                )

        def emit_proj(j, sb, f):
            ss = slice(sb * 128, (sb + 1) * 128)
            sf = slice(f * 512, (f + 1) * 512)
            yp = psp.tile([128, 512], f32, tag="mm512", bufs=2, name="yproj")
            for r in range(PAIRS):
                nc.tensor.matmul(yp, lhsT=cat[:, r, ss], rhs=wp_sb[:, r, sf],
                                 start=(r == 0), stop=(r == PAIRS - 1))
            ys = work.tile([128, 512], f32, tag="ys", bufs=3)
            nc.vector.tensor_copy(ys, yp)
            nc.sync.dma_start(out=y_d[ss, sf], in_=ys)

        def attn_pipeline(j, r):
            """sc leads pv-pairs; returns list of item closures."""
            nt = 4 * j + 4
            items = []
            for ti in range(nt):
                items.append(lambda j=j, r=r, ti=ti: emit_sc(j, r, ti))
                if ti >= 3 and ti % 2 == 1:
                    items.append(
                        lambda j=j, r=r, tp=(ti - 3) // 2: emit_pvp(j, r, tp)
                    )
            items.append(lambda j=j, r=r: emit_pvp(j, r, nt // 2 - 1))
            items.append(lambda j=j, r=r: emit_norm(j, r))
            return items

        def interleave(main, filler):
            """Emit main items with filler spread evenly between them."""
            if not main:
                for g in filler:
                    g()
                return
            k = len(filler) / (len(main) + 1)
            fi = 0
            for n, g in enumerate(main):
                g()
                want = int((n + 1) * k)
                while fi < min(want, len(filler)):
                    filler[fi]()
                    fi += 1
            while fi < len(filler):
                filler[fi]()
                fi += 1

        # ---------------- emission ----------------
        # j=0 QKV interleaved by r to match the split weight DMA arrivals
        for r in range(PAIRS):
            emit_q(0, r)
            emit_warm()
            emit_k(0, r)
            emit_warm()
        for ii in range(4):
            emit_v(0, ii)
            emit_warm()

        for j in range(SCH):
            # K(j)/V(j)/Q(j) were already emitted as chunk j-1 fillers (or in
            # the j=0 preamble), so every chunk's attention starts immediately.
            main = []
            for r in range(PAIRS):
                main += attn_pipeline(j, r)
            filler = []
            if j + 1 < SCH:
                for r in range(PAIRS):
                    filler.append(lambda j=j, r=r: emit_q(j + 1, r))
                    filler.append(lambda j=j, r=r: emit_k(j + 1, r))
                for ii in range(4):
                    filler.append(lambda j=j, ii=ii: emit_v(j + 1, ii))
            if j > 0:
                for sb in range(4 * (j - 1), 4 * j):
                    for f in range(2):
                        filler.append(
                            lambda sb=sb, f=f, j=j: emit_proj(j - 1, sb, f)
                        )
            for _ in range(0 if j < 2 else 4 * j):
                filler.append(emit_warm)
            interleave(main, filler)

        wps2 = psp.tile([128, 512], f32, tag="mm512", bufs=2, name="warm2")
        for i in range(24):
            nc.tensor.matmul(wps2, lhsT=dumm[:, 0:128], rhs=dumm[:, 128:640],
                             start=(i == 0), stop=(i == 23))
        for sb in range(12, 16):
            for f in range(2):
                emit_proj(3, sb, f)

        if DEBUG_DUMPS:
            for nm, t in [("qt", qt), ("kt", kt), ("va", va), ("cat", cat),
                          ("rdb", rdb), ("mask", mask_sb)]:
                nc.sync.dma_start(out=dbg_d[nm], in_=t)
            nc.sync.dma_start(out=dbg_dn, in_=dn)

    nc.compile()
    return nc


def get_nc():
    if "nc" not in _CACHE:
        _CACHE["nc"] = _build()
    return _CACHE["nc"]


def prep_core_inputs(x, Wq, bq, Wk, Wv, Wp, core):
    """Pack the full-model inputs into one core's input map."""
    b, g = core // 2, core % 2
    heads = list(range(g * 8, g * 8 + 8))

    def pack_qk(W):  # [H,E,D] -> local [E, 512] -> [128, 4, 8, 128]
        Wl = np.concatenate([W[h] for h in heads], axis=1)
        return host_round(
            Wl.reshape(ET, 128, PAIRS, 128).transpose(1, 2, 0, 3)
        )

    def pack_v(W):  # [128, 8, 512] (e-tiles on partitions)
        Wl = np.concatenate([W[h] for h in heads], axis=1)
        return host_round(Wl.reshape(ET, 128, 512).transpose(1, 0, 2))

    wp_l = host_round(
        Wp[g * 512:(g + 1) * 512].reshape(PAIRS, 128, E).transpose(1, 0, 2)
    )

    bq_l = np.stack(
        [
            np.concatenate([bq[heads[2 * r]], bq[heads[2 * r + 1]]])
            for r in range(PAIRS)
        ],
        axis=1,
    ).astype(np.float32)

    p = np.arange(128)[:, None]
    u = np.arange(512)[None, :]
    mask = host_round((u >= p).astype(np.float32))
    sel = np.zeros((65, 128), np.float32)
    sel[0, 0:64] = 1.0
    sel[64, 64:128] = 1.0

    return {
        "xt": host_round(x[b].T.reshape(ET, 128, S)),
        "wq": pack_qk(Wq),
        "wk": pack_qk(Wk),
        "wv": pack_v(Wv),
        "wp": wp_l,
        "bq": bq_l,
        "mask": mask,
        "sel": host_round(sel),
        "dumm": host_round(np.ones((128, 640), np.float32)),
    }


def kernel(**inputs):
    from concourse.bass_utils import run_bass_kernel_spmd

    args = {k: np.asarray(v, np.float32) for k, v in inputs.items()}
    nc = get_nc()
    in_maps = [
        prep_core_inputs(
            args["x"], args["Wq"], args["bq"], args["Wk"], args["Wv"],
            args["Wp"], c,
        )
        for c in range(NCORES)
    ]
    res = run_bass_kernel_spmd(nc, in_maps, core_ids=list(range(NCORES)))
    parts = [r["y"] for r in res.results]
    out = np.stack([parts[2 * b] + parts[2 * b + 1] for b in range(B)])
    # bv passes through attention unchanged (softmax weights sum to 1);
    # add its projection on the host along with bp.
    bias = args["bp"] + args["bv"].reshape(-1) @ args["Wp"]
    return (out + bias[None, None, :]).astype(np.float32)


# revision 22
# speedup vs baseline: 1.0526x; 1.0526x over previous
"""Multi-head causal attention (B=4, S=2048, E=1024, H=16, D=64) on 8 trn2 cores.

Sharding: core c handles batch b = c//2 and head-group g = c%2 (8 heads each).
Each core computes its partial output projection over its 512 local concat
columns; the host sums the two partials per batch and adds bp.

Math simplifications (exact):
  - bk is dropped: (q+bq)@bk is constant over the key index, so it cancels
    in the softmax.
  - bv is dropped on-device: softmax weights sum to 1, so bv passes through
    attention unchanged; the host adds bv_flat @ Wp into the bp term.

Layout strategy (per core):
  - x pre-transposed on host: xT [8, 128, S] (e on partitions).
  - Q^T, K^T computed as [d, s] (d on partitions, 2 heads per 128-partition
    pair tile) so scores come out transposed: scoresT [t, s]. The two heads'
    score matmuls are K=64 row-tiled (tile_position auto-derived) and run
    concurrently on the two halves of the PE array.
  - V kept natural [t, d] with a ones column appended per head (padded to 66
    cols), so the PV matmul also produces the softmax denominator as row 64.
  - Softmax: exp on ACT (no max subtraction -- scores are O(1)), causal
    masking via one multiplicative [128,512] (u>=p) mask on DVE, denominator
    pair broadcast across partitions via one K=2 selector matmul, reciprocal
    on DVE, normalize into concat^T, output projection from concat^T.

Schedule strategy:
  - DMA priority: wq/wk are split per head-pair r so the first projection
    matmuls start ~4us in; 20 dummy warmup matmuls keep the PE HAM clock
    gate at full rate while inputs stream.
  - scores->exp->PV is software-pipelined 2 blocks deep so the ACT exp
    latency never stalls the PE queue head; proj is emitted in half-items
    as filler.
"""

import numpy as np

B, S, E, H, D = 4, 2048, 1024, 16, 64
NCORES = 8
PAIRS = 4  # head pairs per core (8 heads)
ET = 8  # e-tiles of 128 for the contraction over E
SCH = 4  # s-chunks of 512
VW = 66  # V columns per head: 64 d + 1 ones + 1 pad
SCALE = float(D) ** -0.5
WARMUP_MMS = 10

MM_DTYPE = "bfloat16"
PV_FP8 = True  # fp8e4 DoubleRow for the PV matmuls (2 t-blocks per matmul)
DEBUG_DUMPS = False

_CACHE = {}


def host_round(a):
    a = np.ascontiguousarray(a, np.float32)
    if MM_DTYPE == "bfloat16":
        import ml_dtypes

        return a.astype(ml_dtypes.bfloat16)
    return a


def _build():
    import concourse.tile as tile
    from concourse import bacc, mybir
    from contextlib import ExitStack

    f32 = mybir.dt.float32
    mdt = getattr(mybir.dt, MM_DTYPE)
    f8 = mybir.dt.float8e4
    pdt = f8 if PV_FP8 else mdt
    AF = mybir.ActivationFunctionType

    nc = bacc.Bacc("TRN2", target_bir_lowering=False, debug=False, num_devices=NCORES)

    xt_d = nc.dram_tensor("xt", [8, 128, S], mdt, kind="ExternalInput").ap()
    wq_d = nc.dram_tensor("wq", [128, PAIRS, ET, 128], mdt, kind="ExternalInput").ap()
    wk_d = nc.dram_tensor("wk", [128, PAIRS, ET, 128], mdt, kind="ExternalInput").ap()
    wv_d = nc.dram_tensor("wv", [128, ET, 512], mdt, kind="ExternalInput").ap()
    wp_d = nc.dram_tensor("wp", [128, PAIRS, E], mdt, kind="ExternalInput").ap()
    bq_d = nc.dram_tensor("bq", [128, PAIRS], f32, kind="ExternalInput").ap()
    mask_d = nc.dram_tensor("mask", [128, 512], mdt, kind="ExternalInput").ap()
    dumm_d = nc.dram_tensor("dumm", [128, 640], mdt, kind="ExternalInput").ap()
    sel_d = nc.dram_tensor("sel", [65, 128], mdt, kind="ExternalInput").ap()
    y_d = nc.dram_tensor("y", [S, E], f32, kind="ExternalOutput").ap()
    if DEBUG_DUMPS:
        dbg_dn = nc.dram_tensor("dbg_dn", [65, 512], f32,
                                kind="ExternalOutput").ap()
        dbg_d = {
            nm: nc.dram_tensor(f"dbg_{nm}", shp, mdt, kind="ExternalOutput").ap()
            for nm, shp in [
                ("qt", [128, PAIRS, S]), ("kt", [128, PAIRS, S]),
                ("va", [128, 16, 8 * VW]), ("cat", [128, PAIRS, S]),
                ("rdb", [65, 512]), ("mask", [128, 512]),
            ]
        }

    with tile.TileContext(nc) as tc, ExitStack() as ctx:
        pers = ctx.enter_context(tc.tile_pool(name="pers", bufs=1))
        work = ctx.enter_context(tc.tile_pool(name="work", bufs=1))
        psp = ctx.enter_context(tc.tile_pool(name="psp", bufs=1, space="PSUM"))

        qt = pers.tile([128, PAIRS, S], mdt)  # Q^T pair tiles
        kt = pers.tile([128, PAIRS, S], mdt)  # K^T pair tiles
        # V (+ones col), indexed [pair, ko]; chunk-0 PV runs in bf16 (short
        # prefixes can't average out fp8 noise), chunks 1-3 in fp8 DoubleRow
        if PV_FP8:
            va8 = pers.tile([128, 8, 2, 8 * VW], f8)
            va16 = pers.tile([128, 2, 2, 8 * VW], mdt)
        else:
            va16 = pers.tile([128, 8, 2, 8 * VW], mdt)
            va8 = None
        cat = pers.tile([128, PAIRS, S], mdt)  # concat^T
        bq_sb = pers.tile([128, PAIRS], f32)
        sel = pers.tile([65, 128], mdt)  # denom pair-broadcast selector
        dn = pers.tile([65, 512], f32)  # denoms staged at partitions 0 / 64
        rd32 = pers.tile([65, 512], f32)
        rdb = pers.tile([65, 512], mdt)
        wq_sb = pers.tile([128, PAIRS, ET, 128], mdt)
        wk_sb = pers.tile([128, PAIRS, ET, 128], mdt)
        wv_sb = pers.tile([128, ET, 512], mdt)
        wp_sb = pers.tile([128, PAIRS, E], mdt)
        mask_sb = pers.tile([128, 512], mdt)
        mask8 = pers.tile([128, 512], f8, name="mask8") if PV_FP8 else None
        dumm = pers.tile([128, 640], mdt)
        xts = [work.tile([128, ET, 512], mdt, tag=f"xt{j}", name=f"xt{j}")
               for j in range(SCH)]

        # memsets first: no deps, run immediately
        nc.vector.memset(dn, 1.0)
        nc.vector.memset(rd32, 0.0)
        nc.vector.memset(rdb, 0.0)

        xt_view = xt_d.rearrange("e p s -> p e s")
        # DMA priority order. sync queue: what the first Q matmuls need;
        # gpsimd queue: K weights then the bulk.
        nc.sync.dma_start(out=dumm, in_=dumm_d)
        nc.sync.dma_start(out=wq_sb[:, 0], in_=wq_d[:, 0])
        nc.sync.dma_start(out=xts[0], in_=xt_view[:, :, 0:512])
        nc.gpsimd.dma_start(out=bq_sb, in_=bq_d)
        nc.gpsimd.dma_start(out=sel, in_=sel_d)
        # guard: a writer into wk_sb that depends on xt0 holds the wk DMA
        # (WAW) and the rest of the gpsimd FIFO until the critical xt0 lands,
        # so the first Q matmuls get the full HBM bandwidth.
        nc.vector.tensor_copy(wk_sb[0:1, 0, 0, 0:2], xts[0][0:1, 0, 0:2])
        nc.gpsimd.dma_start(out=wk_sb[:, 0], in_=wk_d[:, 0])
        for r in range(1, PAIRS):
            nc.sync.dma_start(out=wq_sb[:, r], in_=wq_d[:, r])
            nc.gpsimd.dma_start(out=wk_sb[:, r], in_=wk_d[:, r])
        nc.sync.dma_start(out=mask_sb, in_=mask_d)
        if PV_FP8:
            nc.vector.tensor_copy(mask8, mask_sb)
        nc.gpsimd.dma_start(out=wv_sb, in_=wv_d)
        for j in range(1, SCH):
            nc.sync.dma_start(out=xts[j], in_=xt_view[:, :, j * 512:(j + 1) * 512])
        nc.gpsimd.dma_start(out=wp_sb, in_=wp_d)

        # PE warmup: keep the HAM clock gate hot while inputs stream.
        # One accumulation group -> no WAR semaphores between members, so
        # the matmuls stream back-to-back (a full HAM busy window).
        wps = psp.tile([128, 512], f32, tag="mm512", bufs=2, name="warm")
        for i in range(WARMUP_MMS):
            nc.tensor.matmul(wps, lhsT=dumm[:, 0:128], rhs=dumm[:, 128:640],
                             start=(i == 0), stop=(i == WARMUP_MMS - 1))

        # ---------------- work-item emitters ----------------
        def emit_warm(n=2):
            # dummy matmuls into the unused partitions 96-127 of the o0
            # PSUM bank: soak PE idle in ACT-bound or DMA-bound stretches so
            # the HAM clock gate stays at 8/8, with no WAR on real tiles
            t = last_o[0]
            for i in range(n):
                nc.tensor.matmul(t[96:128, :], lhsT=dumm[:, 96:128],
                                 rhs=dumm[:, 128:640], start=(i == 0),
                                 stop=(i == n - 1), skip_group_check=True,
                                 tile_position=(0, 96))

        def emit_q(j, r):
            sjl = slice(j * 512, (j + 1) * 512)
            ps = psp.tile([128, 512], f32, tag="mm512", bufs=2, name="qps")
            for et in range(ET):
                nc.tensor.matmul(ps, lhsT=wq_sb[:, r, et], rhs=xts[j][:, et],
                                 start=(et == 0), stop=(et == ET - 1))
            nc.vector.tensor_scalar_add(qt[:, r, sjl], ps, bq_sb[:, r:r + 1])

        def emit_k(j, r):
            sjl = slice(j * 512, (j + 1) * 512)
            ps = psp.tile([128, 512], f32, tag="mm512", bufs=2, name="kps")
            for et in range(ET):
                nc.tensor.matmul(ps, lhsT=wk_sb[:, r, et], rhs=xts[j][:, et],
                                 start=(et == 0), stop=(et == ET - 1))
            nc.vector.tensor_copy(kt[:, r, sjl], ps)

        def emit_v(j, ii):
            i = 4 * j + ii
            si = slice(ii * 128, (ii + 1) * 128)
            ps = psp.tile([128, 512], f32, tag="mm512", bufs=2, name="vps")
            for et in range(ET):
                nc.tensor.matmul(ps, lhsT=xts[j][:, et, si], rhs=wv_sb[:, et],
                                 start=(et == 0), stop=(et == ET - 1))
            tgts = []
            if PV_FP8:
                tgts.append(va8[:, i // 2, i % 2, :])
                if j == 0:
                    tgts.append(va16[:, i // 2, i % 2, :])
            else:
                tgts.append(va16[:, i // 2, i % 2, :])
            for t in tgts:
                va_i = t.rearrange("p (h c) -> p h c", c=VW)
                nc.vector.tensor_copy(
                    va_i[:, :, 0:64], ps.rearrange("p (h d) -> p h d", d=64)
                )
                nc.vector.memset(va_i[:, :, 64:65], 1.0)
                nc.vector.memset(va_i[:, :, 65:66], 0.0)

        attn_pr = {}
        attn_out = {}
        last_o = [None]

        def emit_sc(j, r, ti):
            v = max(ti - 4 * j, 0)
            sjv = slice(j * 512 + 128 * v, (j + 1) * 512)
            tis = slice(ti * 128, (ti + 1) * 128)
            scp = psp.tile([128, 2, 512], f32, tag="sc", bufs=2)
            for hh in range(2):
                po = hh * 64
                nc.tensor.matmul(
                    scp[:, hh, 128 * v:],
                    lhsT=kt[po:po + 64, r, tis],
                    rhs=qt[po:po + 64, r, sjv],
                    start=True, stop=True,
                )
            tp, ko = ti // 2, ti % 2
            fp8c = PV_FP8 and j > 0
            if ko == 0:
                if fp8c:
                    attn_pr[(j, r, tp)] = work.tile(
                        [128, 2, 2, 512], f8, tag="pr", bufs=6, name="pr8"
                    )
                else:
                    attn_pr[(j, r, tp)] = work.tile(
                        [128, 2, 2, 512], mdt, tag="pr16", bufs=3, name="pr16"
                    )
            pr = attn_pr[(j, r, tp)]
            nc.scalar.activation(
                pr[:, :, ko, 128 * v:], scp[:, :, 128 * v:], AF.Exp,
                scale=SCALE
            )
            if v or ti == 4 * j:
                msk = mask8 if fp8c else mask_sb
                for hh in range(2):
                    nc.vector.tensor_mul(
                        pr[:, hh, ko, 128 * v:],
                        pr[:, hh, ko, 128 * v:],
                        msk[:, 0:512 - 128 * v],
                    )
            if fp8c and ko == 1 and v >= 1:
                # diagonal pair hole: the odd block's [128(v-1), 128v) slice
                # is streamed by the pair matmul but must contribute zero
                nc.vector.memset(pr[:, :, 1, 128 * (v - 1):128 * v], 0.0)

        def emit_pvp(j, r, tp):
            np2 = 2 * j + 2
            if tp == 0:
                attn_out[(j, r)] = [
                    psp.tile([128, 512], f32, tag=f"o{hh}", bufs=1,
                             name=f"outp{hh}")
                    for hh in range(2)
                ]
                last_o[0] = attn_out[(j, r)][0]
            outps = attn_out[(j, r)]
            pr = attn_pr.pop((j, r, tp))
            vA = max(2 * tp - 4 * j, 0)
            fp8c = PV_FP8 and j > 0
            for hh in range(2):
                h = 2 * r + hh
                if fp8c:
                    nc.tensor.matmul(
                        outps[hh][0:VW, 128 * vA:],
                        lhsT=va8[:, tp, :, h * VW:(h + 1) * VW],
                        rhs=pr[:, hh, :, 128 * vA:],
                        perf_mode=mybir.MatmulPerfMode.DoubleRow,
                        start=(tp == 0), stop=(tp == np2 - 1),
                    )
                else:
                    for ko in range(2):
                        v = max(2 * tp + ko - 4 * j, 0)
                        nc.tensor.matmul(
                            outps[hh][0:VW, 128 * v:],
                            lhsT=va16[:, tp, ko, h * VW:(h + 1) * VW],
                            rhs=pr[:, hh, ko, 128 * v:],
                            start=(tp == 0 and ko == 0),
                            stop=(tp == np2 - 1 and ko == 1),
                        )

        def emit_norm(j, r):
            outps = attn_out.pop((j, r))
            sjl = slice(j * 512, (j + 1) * 512)
            # PSUM evacuation first: releases the o-banks for the next
            # head pair's PV accumulation as early as possible
            osbs = []
            for hh in range(2):
                osb = work.tile([65, 512], mdt, tag="osb", bufs=4)
                nc.vector.tensor_copy(osb, outps[hh][0:65, :])
                osbs.append(osb)
            # denominators (row 64) -> reciprocal -> pair-broadcast matmul.
            # Full-tile DVE ops from base partition 0: custom-DVE ops
            # misbehave at base partition 64 on HW (rows 1-63 are 1.0 and
            # sel zeroes them in the matmul).
            for hh, po in ((0, 0), (1, 64)):
                nc.vector.tensor_copy(dn[po:po + 1], osbs[hh][64:65, :])
            nc.vector.reciprocal_approx_fast(rd32, dn)
            nc.vector.tensor_copy(rdb, rd32)
            bcp = psp.tile([128, 512], f32, tag="mm512", bufs=2, name="bcst")
            nc.tensor.matmul(bcp, lhsT=sel, rhs=rdb, start=True, stop=True)
            for hh in range(2):
                po = hh * 64
                nc.vector.tensor_mul(
                    cat[po:po + 64, r, sjl], osbs[hh][0:64, :],
                    bcp[po:po + 64, :]
                )

        def emit_proj(j, sb, f):
            ss = slice(sb * 128, (sb + 1) * 128)
            sf = slice(f * 512, (f + 1) * 512)
            yp = psp.tile([128, 512], f32, tag="mm512", bufs=2, name="yproj")
            for r in range(PAIRS):
                nc.tensor.matmul(yp, lhsT=cat[:, r, ss], rhs=wp_sb[:, r, sf],
                                 start=(r == 0), stop=(r == PAIRS - 1))
            ys = work.tile([128, 512], f32, tag="ys", bufs=3)
            nc.vector.tensor_copy(ys, yp)
            nc.sync.dma_start(out=y_d[ss, sf], in_=ys)

        def attn_pipeline(j, r):
            """sc leads pv-pairs; returns list of item closures."""
            nt = 4 * j + 4
            items = []
            for ti in range(nt):
                items.append(lambda j=j, r=r, ti=ti: emit_sc(j, r, ti))
                if ti >= 3 and ti % 2 == 1:
                    items.append(
                        lambda j=j, r=r, tp=(ti - 3) // 2: emit_pvp(j, r, tp)
                    )
            items.append(lambda j=j, r=r: emit_pvp(j, r, nt // 2 - 1))
            items.append(lambda j=j, r=r: emit_norm(j, r))
            return items

        def interleave(main, filler):
            """Emit main items with filler spread evenly between them."""
            if not main:
                for g in filler:
                    g()
                return
            k = len(filler) / (len(main) + 1)
            fi = 0
            for n, g in enumerate(main):
                g()
                want = int((n + 1) * k)
                while fi < min(want, len(filler)):
                    filler[fi]()
                    fi += 1
            while fi < len(filler):
                filler[fi]()
                fi += 1

        # ---------------- emission ----------------
        warmO = psp.tile([128, 512], f32, tag="o0", bufs=1, name="warmO")
        last_o[0] = warmO
        # j=0 QKV interleaved by r to match the split weight DMA arrivals
        for r in range(PAIRS):
            emit_q(0, r)
            emit_warm()
            emit_k(0, r)
            emit_warm()
        for ii in range(4):
            emit_v(0, ii)
            emit_warm()

        for j in range(SCH):
            # K(j)/V(j)/Q(j) were already emitted as chunk j-1 fillers (or in
            # the j=0 preamble), so every chunk's attention starts immediately.
            main = []
            for r in range(PAIRS):
                main += attn_pipeline(j, r)
            filler = []
            if j + 1 < SCH:
                for r in range(PAIRS):
                    filler.append(lambda j=j, r=r: emit_q(j + 1, r))
                    filler.append(lambda j=j, r=r: emit_k(j + 1, r))
                for ii in range(4):
                    filler.append(lambda j=j, ii=ii: emit_v(j + 1, ii))
            # proj is deferrable PE work: schedule it into the late,
            # ACT-bound chunks (j=2 gets chunk 0, j=3 gets chunks 1+2)
            pjs = {2: [0], 3: [1, 2]}.get(j, [])
            for pj in pjs:
                for sb in range(4 * pj, 4 * pj + 4):
                    for f in range(2):
                        filler.append(
                            lambda sb=sb, f=f, pj=pj: emit_proj(pj, sb, f)
                        )
            if j == 3:
                for _ in range(20):
                    filler.append(lambda: emit_warm(3))
            interleave(main, filler)

        emit_warm(24)
        for sb in range(12, 16):
            for f in range(2):
                emit_proj(3, sb, f)

        if DEBUG_DUMPS:
            for nm, t in [("qt", qt), ("kt", kt), ("va", va), ("cat", cat),
                          ("rdb", rdb), ("mask", mask_sb)]:
                nc.sync.dma_start(out=dbg_d[nm], in_=t)
            nc.sync.dma_start(out=dbg_dn, in_=dn)

    nc.compile()
    return nc


def get_nc():
    if "nc" not in _CACHE:
        _CACHE["nc"] = _build()
    return _CACHE["nc"]


def prep_core_inputs(x, Wq, bq, Wk, Wv, Wp, core):
    """Pack the full-model inputs into one core's input map."""
    b, g = core // 2, core % 2
    heads = list(range(g * 8, g * 8 + 8))

    def pack_qk(W):  # [H,E,D] -> local [E, 512] -> [128, 4, 8, 128]
        Wl = np.concatenate([W[h] for h in heads], axis=1)
        return host_round(
            Wl.reshape(ET, 128, PAIRS, 128).transpose(1, 2, 0, 3)
        )

    def pack_v(W):  # [128, 8, 512] (e-tiles on partitions)
        Wl = np.concatenate([W[h] for h in heads], axis=1)
        return host_round(Wl.reshape(ET, 128, 512).transpose(1, 0, 2))

    wp_l = host_round(
        Wp[g * 512:(g + 1) * 512].reshape(PAIRS, 128, E).transpose(1, 0, 2)
    )

    bq_l = np.stack(
        [
            np.concatenate([bq[heads[2 * r]], bq[heads[2 * r + 1]]])
            for r in range(PAIRS)
        ],
        axis=1,
    ).astype(np.float32)

    p = np.arange(128)[:, None]
    u = np.arange(512)[None, :]
    mask = host_round((u >= p).astype(np.float32))
    sel = np.zeros((65, 128), np.float32)
    sel[0, 0:64] = 1.0
    sel[64, 64:128] = 1.0

    return {
        "xt": host_round(x[b].T.reshape(ET, 128, S)),
        "wq": pack_qk(Wq),
        "wk": pack_qk(Wk),
        "wv": pack_v(Wv),
        "wp": wp_l,
        "bq": bq_l,
        "mask": mask,
        "sel": host_round(sel),
        "dumm": host_round(np.ones((128, 640), np.float32)),
    }


def kernel(**inputs):
    from concourse.bass_utils import run_bass_kernel_spmd

    args = {k: np.asarray(v, np.float32) for k, v in inputs.items()}
    nc = get_nc()
    in_maps = [
        prep_core_inputs(
            args["x"], args["Wq"], args["bq"], args["Wk"], args["Wv"],
            args["Wp"], c,
        )
        for c in range(NCORES)
    ]
    res = run_bass_kernel_spmd(nc, in_maps, core_ids=list(range(NCORES)))
    parts = [r["y"] for r in res.results]
    out = np.stack([parts[2 * b] + parts[2 * b + 1] for b in range(B)])
    # bv passes through attention unchanged (softmax weights sum to 1);
    # add its projection on the host along with bp.
    bias = args["bp"] + args["bv"].reshape(-1) @ args["Wp"]
    return (out + bias[None, None, :]).astype(np.float32)
